# revision 9
# baseline (speedup 1.0000x reference)
"""Trainium2 Bass kernel for llama-style attention block (B=4, S=1024, D=4096, H=32).

Strategy: tensor-parallel over heads across 8 NeuronCores (4 heads/core).
 - Host marshals inputs: x transposed to [D, T] (contraction dim on
   partitions), per-core weight slices pre-transposed, q/k weight rows
   deinterleaved (even/odd RoPE pairs -> partition blocks [0:64]/[64:128]),
   everything matmul-facing cast to bf16.
 - Device per core: QKV projections (PE, fp32 accum) -> RoPE (DVE) ->
   attention in transposed layout S^T[k,q]: per k-tile the scores stream
   only the live causal q-range [128*kt, S); the additive mask is applied
   only on the 128-wide diagonal block; softmax denominators come from a
   DVE sum over k-tiles followed by one all-ones matmul (partition
   reduction) and a fast-approx reciprocal; P@V accumulates live ranges
   into two PSUM banks -> per-batch AllGather of context (heads) ->
   output projection slice -> y columns.
 - Host concatenates the 8 per-core y column slices.

kernel(**inputs) takes the full unsharded inputs as in reference.setup_inputs()
and returns the full [4, 1024, 4096] float32 output.
"""

import math
import sys

import numpy as np
import ml_dtypes

sys.path.insert(0, "/opt/trn_rl_repo")

import concourse.bass as bass  # noqa: E402
import concourse.bass_isa as bass_isa  # noqa: E402
import concourse.mybir as mybir  # noqa: E402
import concourse.tile as tile  # noqa: E402
from concourse import bacc  # noqa: E402
from concourse.bass_utils import run_bass_kernel_spmd  # noqa: E402

P = 128
B, S, D, H = 4, 1024, 4096, 32
T = B * S
HD = 128
NCORES = 8
HPC = H // NCORES          # heads per core = 4
CW = HPC * HD              # per-core width = 512
NDK = D // P               # 32 contraction tiles
TCH = 256                  # token chunk in projection phase
NKT = S // P               # 8 k tiles per batch

MM = mybir.dt.bfloat16     # matmul operand dtype
F32 = mybir.dt.float32
BF16 = ml_dtypes.bfloat16

AG_GROUPS = [list(range(NCORES))]


def build_program():
    """Specialized to causal start_pos=0: per k-tile kt, queries in
    [128*kt, S) are live; the [128*kt, 128*(kt+1)) block needs the mask."""
    nc = bacc.Bacc("TRN2", target_bir_lowering=False, debug=False,
                   num_devices=NCORES)

    xT = nc.dram_tensor("xT", [D, T], MM, kind="ExternalInput").ap()
    wqT = nc.dram_tensor("wqT", [D, CW], MM, kind="ExternalInput").ap()
    wkT = nc.dram_tensor("wkT", [D, CW], MM, kind="ExternalInput").ap()
    wvT = nc.dram_tensor("wvT", [D, CW], MM, kind="ExternalInput").ap()
    woT = nc.dram_tensor("woT", [D, CW], MM, kind="ExternalInput").ap()
    maskD = nc.dram_tensor("maskD", [P, NKT * P], MM, kind="ExternalInput").ap()
    cq = nc.dram_tensor("cq", [HD // 2, S], F32, kind="ExternalInput").ap()
    sq = nc.dram_tensor("sq", [HD // 2, S], F32, kind="ExternalInput").ap()
    ck = nc.dram_tensor("ck", [HD // 2, S], F32, kind="ExternalInput").ap()
    sk = nc.dram_tensor("sk", [HD // 2, S], F32, kind="ExternalInput").ap()
    y = nc.dram_tensor("y", [T, CW], F32, kind="ExternalOutput").ap()

    qT_d = nc.dram_tensor("qT_d", [CW, T], MM).ap()
    kT_d = nc.dram_tensor("kT_d", [CW, T], MM).ap()
    v_d = nc.dram_tensor("v_d", [T, CW], MM).ap()
    bounce = [nc.dram_tensor(f"bnc{i}", [CW, S], MM).ap()
              for i in range(B)]
    ctxT = [nc.dram_tensor(f"ctxT{i}", [D, S], MM,
                           addr_space="Shared").ap()
            for i in range(B)]

    sub = mybir.AluOpType.subtract
    add = mybir.AluOpType.add
    mult = mybir.AluOpType.mult
    Exp = mybir.ActivationFunctionType.Exp

    with tile.TileContext(nc) as tc:
        # long-lived pools: batch-0 attention prefetch + constants
        with tc.tile_pool(name="p0", bufs=1) as p0, \
             tc.tile_pool(name="mpool", bufs=1) as mpool:

            qb0 = p0.tile([P, HPC, S], MM)
            kb0 = p0.tile([P, HPC, S], MM)
            vb0 = p0.tile([P, NKT, CW], MM)
            mask_sb = mpool.tile([P, NKT, P], MM)
            ones_sb = mpool.tile([P, P], MM)

            # ---------------- Phase A: projections + RoPE ----------------
            with tc.tile_pool(name="wpool", bufs=1) as wpool, \
                 tc.tile_pool(name="cspool", bufs=1) as cspool, \
                 tc.tile_pool(name="xpool", bufs=2) as xpool, \
                 tc.tile_pool(name="psa", bufs=4, space="PSUM") as psa, \
                 tc.tile_pool(name="stga", bufs=4) as stga, \
                 tc.tile_pool(name="tmpa", bufs=2) as tmpa:

                # first x chunk + head-0 q weights in interleaved pieces on
                # two DMA queues so the first accumulation group starts early
                x_first = xpool.tile([P, NDK, TCH], MM, tag="x")
                xTr = xT[:, 0:TCH].rearrange("(o p) t -> p o t", p=P)
                wq_sb = wpool.tile([P, NDK, CW], MM)
                wk_sb = wpool.tile([P, NDK, CW], MM)
                wv_sb = wpool.tile([P, NDK, CW], MM)
                wqr = wqT[:, 0:HD].rearrange("(o p) m -> p o m", p=P)
                wkr = wkT[:, 0:HD].rearrange("(o p) m -> p o m", p=P)
                for pc in range(8):
                    dsl = slice(pc * 4, (pc + 1) * 4)
                    nc.sync.dma_start(wq_sb[:, dsl, 0:HD], wqr[:, dsl, :])
                    nc.scalar.dma_start(x_first[:, dsl, :], xTr[:, dsl, :])
                nc.sync.dma_start(wk_sb[:, :, 0:HD], wkr)

                cq_sb = cspool.tile([HD // 2, S], F32)
                sq_sb = cspool.tile([HD // 2, S], F32)
                ck_sb = cspool.tile([HD // 2, S], F32)
                sk_sb = cspool.tile([HD // 2, S], F32)
                nc.sync.dma_start(cq_sb, cq)
                nc.sync.dma_start(sq_sb, sq)
                nc.sync.dma_start(ck_sb, ck)
                nc.sync.dma_start(sk_sb, sk)

                for h in range(1, HPC):
                    hs = slice(h * HD, (h + 1) * HD)
                    nc.sync.dma_start(
                        wq_sb[:, :, hs],
                        wqT[:, hs].rearrange("(o p) m -> p o m", p=P))
                    nc.sync.dma_start(
                        wk_sb[:, :, hs],
                        wkT[:, hs].rearrange("(o p) m -> p o m", p=P))
                nc.sync.dma_start(wv_sb, wvT.rearrange("(o p) m -> p o m", p=P))

                for tch in range(T // TCH):
                    t0 = tch * TCH
                    s0 = t0 % S
                    if tch == 0:
                        x_sb = x_first
                    else:
                        x_sb = xpool.tile([P, NDK, TCH], MM, tag="x")
                        nc.sync.dma_start(
                            x_sb,
                            xT[:, t0:t0 + TCH].rearrange("(o p) t -> p o t",
                                                         p=P))

                    # q/k for the 4 local heads; RoPE on psum eviction
                    for h in range(HPC):
                        for wsb, c_sb, s_sb, dst in (
                                (wq_sb, cq_sb, sq_sb, qT_d),
                                (wk_sb, ck_sb, sk_sb, kT_d)):
                            ps = psa.tile([P, TCH], F32, tag="qk")
                            for dk in range(NDK):
                                nc.tensor.matmul(
                                    ps, lhsT=wsb[:, dk, h * HD:(h + 1) * HD],
                                    rhs=x_sb[:, dk, :],
                                    start=(dk == 0), stop=(dk == NDK - 1))
                            a = ps[0:HD // 2]
                            bb = ps[HD // 2:P]
                            cc = c_sb[:, s0:s0 + TCH]
                            ss = s_sb[:, s0:s0 + TCH]
                            t1 = tmpa.tile([HD // 2, TCH], F32, tag="t1")
                            t2 = tmpa.tile([HD // 2, TCH], F32, tag="t2")
                            t3 = tmpa.tile([HD // 2, TCH], F32, tag="t3")
                            t4 = tmpa.tile([HD // 2, TCH], F32, tag="t4")
                            out = stga.tile([P, TCH], MM, tag="qkstage")
                            nc.vector.tensor_tensor(t1, a, cc, mult)
                            nc.vector.tensor_tensor(t2, bb, ss, mult)
                            nc.vector.tensor_tensor(out[0:HD // 2], t1, t2, sub)
                            nc.vector.tensor_tensor(t3, a, ss, mult)
                            nc.vector.tensor_tensor(t4, bb, cc, mult)
                            nc.vector.tensor_tensor(out[HD // 2:P], t3, t4, add)
                            nc.sync.dma_start(
                                dst[h * HD:(h + 1) * HD, t0:t0 + TCH], out)

                    # v for the 4 local heads (natural [t, hd] layout);
                    # evict on the otherwise-idle scalar engine
                    for tt in range(TCH // P):
                        ps = psa.tile([P, CW], F32, tag="v")
                        for dk in range(NDK):
                            nc.tensor.matmul(
                                ps, lhsT=x_sb[:, dk, tt * P:(tt + 1) * P],
                                rhs=wv_sb[:, dk, :],
                                start=(dk == 0), stop=(dk == NDK - 1))
                        vo = stga.tile([P, CW], MM, tag="vstage")
                        nc.scalar.copy(vo, ps)
                        nc.sync.dma_start(
                            v_d[t0 + tt * P:t0 + (tt + 1) * P, :], vo)

                    # prefetch batch-0 attention tiles once its q/k/v stored
                    if tch == S // TCH - 1:
                        nc.sync.dma_start(
                            qb0, qT_d[:, 0:S]
                            .rearrange("(h p) t -> p h t", p=P))
                        nc.sync.dma_start(
                            kb0, kT_d[:, 0:S]
                            .rearrange("(h p) t -> p h t", p=P))
                        nc.sync.dma_start(
                            vb0, v_d[0:S, :]
                            .rearrange("(kt p) w -> p kt w", p=P))
                        nc.sync.dma_start(
                            mask_sb,
                            maskD.rearrange("p (kt q) -> p kt q", q=P))
                        nc.any.memset(ones_sb, 1.0)

            # ------------- Phase B/C: attention + AllGather + wo ---------
            with tc.tile_pool(name="qkvp", bufs=2) as qkvp, \
                 tc.tile_pool(name="esp", bufs=2) as esp, \
                 tc.tile_pool(name="accp", bufs=2) as accp, \
                 tc.tile_pool(name="psb", bufs=1, space="PSUM") as psb, \
                 tc.tile_pool(name="tmpb", bufs=4) as tmpb, \
                 tc.tile_pool(name="stgb", bufs=4) as stgb, \
                 tc.tile_pool(name="wop", bufs=1) as wop, \
                 tc.tile_pool(name="cxp", bufs=2) as cxp:

                wo_sb = wop.tile([P, NDK, CW], MM)
                nc.sync.dma_start(wo_sb,
                                  woT.rearrange("(o p) m -> p o m", p=P))

                def load_batch(b):
                    qb = qkvp.tile([P, HPC, S], MM, tag="qb")
                    kb = qkvp.tile([P, HPC, S], MM, tag="kb")
                    vb = qkvp.tile([P, NKT, CW], MM, tag="vb")
                    nc.sync.dma_start(
                        qb, qT_d[:, b * S:(b + 1) * S]
                        .rearrange("(h p) t -> p h t", p=P))
                    nc.sync.dma_start(
                        kb, kT_d[:, b * S:(b + 1) * S]
                        .rearrange("(h p) t -> p h t", p=P))
                    nc.sync.dma_start(
                        vb, v_d[b * S:(b + 1) * S, :]
                        .rearrange("(kt p) w -> p kt w", p=P))
                    return qb, kb, vb

                def pv_mm(st, kt):
                    # P@V matmuls for k-tile kt of the previous slot
                    b, h, qb, kb, vb, es, acc2, ps_o0, ps_o1 = st
                    hs = slice(h * HD, (h + 1) * HD)
                    if kt < 4:
                        lo = P * kt
                        nc.tensor.matmul(
                            ps_o0[:, lo:512], lhsT=vb[:, kt, hs],
                            rhs=es[:, kt, lo:512],
                            start=(kt == 0), stop=(kt == 3),
                            skip_group_check=True)
                    lo2 = max(512, P * kt)
                    nc.tensor.matmul(
                        ps_o1[:, lo2 - 512:512], lhsT=vb[:, kt, hs],
                        rhs=es[:, kt, lo2:S],
                        start=(kt == 0), stop=(kt == NKT - 1),
                        skip_group_check=True)

                def finish_prev(st):
                    # denominators via all-ones MM over kt-pair sums,
                    # fast reciprocal, normalize, stage to bounce
                    b, h, qb, kb, vb, es, acc2, ps_o0, ps_o1 = st
                    ps_d0 = psb.tile([P, 512], F32, tag="d0", bufs=1)
                    ps_d1 = psb.tile([P, 512], F32, tag="d1", bufs=1)
                    nc.tensor.matmul(ps_d0, lhsT=ones_sb,
                                     rhs=acc2[:, 0, 0:512],
                                     start=True, stop=False,
                                     skip_group_check=True)
                    nc.tensor.matmul(ps_d0[:, 256:512], lhsT=ones_sb,
                                     rhs=acc2[:, 1, 256:512],
                                     start=False, stop=True,
                                     skip_group_check=True)
                    for j in range(4):
                        lo = max(512, 256 * j)
                        nc.tensor.matmul(
                            ps_d1[:, lo - 512:512], lhsT=ones_sb,
                            rhs=acc2[:, j, lo:S],
                            start=(j == 0), stop=(j == 3),
                            skip_group_check=True)
                    rec = tmpb.tile([P, S], F32, tag="rec", bufs=2)
                    nc.vector.reciprocal_approx_fast(rec[:, 0:512], ps_d0)
                    nc.vector.reciprocal_approx_fast(rec[:, 512:S], ps_d1)
                    for q2, ps_o in ((0, ps_o0), (1, ps_o1)):
                        ob = stgb.tile([P, 512], MM, tag="ob", bufs=3)
                        nc.vector.tensor_tensor(
                            ob, ps_o, rec[:, q2 * 512:(q2 + 1) * 512], mult)
                        nc.sync.dma_start(
                            bounce[b][h * HD:(h + 1) * HD,
                                      q2 * 512:(q2 + 1) * 512], ob)

                def allgather(i):
                    nc.gpsimd.collective_compute(
                        "AllGather", mybir.AluOpType.bypass,
                        replica_groups=AG_GROUPS,
                        ins=[bounce[i]], outs=[ctxT[i]])

                # flat head pipeline: scores(slot i) interleaved with
                # P@V(slot i-1) so PE never waits on ACT exp evictions
                prev = None
                batch_tiles = {0: (qb0, kb0, vb0)}
                for i in range(B * HPC):
                    b, h = divmod(i, HPC)
                    if h == 3 and b + 1 < B:
                        batch_tiles[b + 1] = load_batch(b + 1)
                    qb, kb, vb = batch_tiles[b]
                    es = esp.tile([P, NKT, S], MM, tag="es")
                    acc2 = accp.tile([P, 4, S], MM, tag="acc2")
                    ps_o0 = ps_o1 = None
                    if prev is not None:
                        ps_o0 = psb.tile([P, 512], F32, tag="o0", bufs=2)
                        ps_o1 = psb.tile([P, 512], F32, tag="o1", bufs=2)
                        prev = prev[:7] + (ps_o0, ps_o1)
                    for kt in range(NKT):
                        lo = P * kt
                        chunks = [(lo, min(lo + 512, S))]
                        if S - lo > 512:
                            chunks.append((lo + 512, S))
                        for (c0, c1) in chunks:
                            n = c1 - c0
                            ps_s = psb.tile([P, 512], F32, tag="sc", bufs=2)
                            nc.tensor.matmul(
                                ps_s[:, 0:n], lhsT=kb[:, h, lo:lo + P],
                                rhs=qb[:, h, c0:c1], start=True, stop=True)
                            if c0 == lo:
                                tmp = tmpb.tile([P, P], F32, tag="dadd")
                                nc.vector.tensor_tensor(
                                    tmp, ps_s[:, 0:P], mask_sb[:, kt, :], add)
                                if n > P:
                                    nc.scalar.activation(
                                        es[:, kt, lo + P:c1],
                                        ps_s[:, P:n], Exp)
                                nc.scalar.activation(
                                    es[:, kt, lo:lo + P], tmp, Exp)
                            else:
                                nc.scalar.activation(
                                    es[:, kt, c0:c1], ps_s[:, 0:n], Exp)
                        if kt % 2 == 1:
                            # gpsimd kt-pair sums for the denominator
                            j = kt // 2
                            a0, a1 = 256 * j, 256 * j + P
                            nc.gpsimd.tensor_scalar_add(
                                acc2[:, j, a0:a1], es[:, kt - 1, a0:a1], 0.0)
                            nc.gpsimd.tensor_tensor(
                                acc2[:, j, a1:S], es[:, kt - 1, a1:S],
                                es[:, kt, a1:S], add)
                        if prev is not None:
                            pv_mm(prev, kt)
                    if prev is not None:
                        finish_prev(prev)
                        if prev[1] == 3:
                            allgather(prev[0])
                    prev = (b, h, qb, kb, vb, es, acc2, None, None)
                # drain the last slot
                ps_o0 = psb.tile([P, 512], F32, tag="o0", bufs=2)
                ps_o1 = psb.tile([P, 512], F32, tag="o1", bufs=2)
                prev = prev[:7] + (ps_o0, ps_o1)
                for kt in range(NKT):
                    pv_mm(prev, kt)
                finish_prev(prev)
                allgather(prev[0])

                # output projection, one batch at a time
                for b in range(B):
                    # paired token tiles: 512B DMA lines on the ctx reads
                    for tt in range(0, S // P, 2):
                        c0 = tt * P
                        cx = cxp.tile([P, NDK, 2 * P], MM, tag="cx")
                        nc.scalar.dma_start(
                            cx, ctxT[b][:, c0:c0 + 2 * P]
                            .rearrange("(o p) t -> p o t", p=P))
                        ps_y0 = psb.tile([P, CW], F32, tag="sc", bufs=2)
                        ps_y1 = psb.tile([P, CW], F32, tag="sc", bufs=2)
                        for dk in range(NDK):
                            nc.tensor.matmul(
                                ps_y0, lhsT=cx[:, dk, 0:P],
                                rhs=wo_sb[:, dk, :],
                                start=(dk == 0), stop=(dk == NDK - 1))
                            nc.tensor.matmul(
                                ps_y1, lhsT=cx[:, dk, P:2 * P],
                                rhs=wo_sb[:, dk, :],
                                start=(dk == 0), stop=(dk == NDK - 1))
                        for j, ps_y in enumerate((ps_y0, ps_y1)):
                            yo = stgb.tile([P, CW], F32, tag="yo", bufs=2)
                            nc.scalar.copy(yo, ps_y)
                            nc.sync.dma_start(
                                y[b * S + (tt + j) * P:
                                  b * S + (tt + j + 1) * P, :], yo)

    nc.compile()
    return nc


_NC_CACHE = {}


def _get_nc():
    if "nc" not in _NC_CACHE:
        _NC_CACHE["nc"] = build_program()
    return _NC_CACHE["nc"]


def _check_causal(maskT_f32):
    """Verify the mask is standard causal (start_pos=0): per k-tile kt,
    queries < 128*kt are fully masked, the diagonal block is the only
    partially-masked one, and everything beyond is unmasked."""
    for kt in range(NKT):
        lo = P * kt
        t = maskT_f32[lo:lo + P, :]
        if lo > 0 and not np.all(t[:, :lo] <= -1e30):
            return False
        if not np.all(t[:, lo + P:] == 0.0):
            return False
    return True


def _prep_inputs(x, freqs_cos, freqs_sin, mask, wq, wk, wv, wo):
    """Host-side sharding/layout marshaling. Returns per-core input maps."""
    x = np.asarray(x, np.float32).reshape(T, D)
    xT = np.ascontiguousarray(x.T.astype(BF16))

    cos = np.asarray(freqs_cos, np.float32)
    sin = np.asarray(freqs_sin, np.float32)
    qscale = 1.0 / math.sqrt(HD)
    cqh = np.ascontiguousarray(cos.T * qscale).astype(np.float32)
    sqh = np.ascontiguousarray(sin.T * qscale).astype(np.float32)
    ckh = np.ascontiguousarray(cos.T).astype(np.float32)
    skh = np.ascontiguousarray(sin.T).astype(np.float32)

    m = np.asarray(mask, np.float32).reshape(S, S)
    mT = np.ascontiguousarray(m.T)
    assert _check_causal(mT), "kernel specialized for causal start_pos=0"
    # diagonal blocks of mask^T, clamped for bf16
    maskDh = np.empty((P, NKT * P), np.float32)
    for kt in range(NKT):
        lo = P * kt
        maskDh[:, lo:lo + P] = mT[lo:lo + P, lo:lo + P]
    maskDb = np.ascontiguousarray(np.maximum(maskDh, -60000.0).astype(BF16))

    # deinterleave RoPE pairs within each head's weight rows: row order
    # [0,2,...,126,1,3,...,127] so pairs land in partition blocks.
    perm = np.concatenate([np.arange(0, HD, 2), np.arange(1, HD, 2)])

    wq = np.asarray(wq, np.float32)
    wk = np.asarray(wk, np.float32)
    wv = np.asarray(wv, np.float32)
    wo = np.asarray(wo, np.float32)

    in_maps = []
    for c in range(NCORES):
        r0, r1 = c * CW, (c + 1) * CW
        wq_c = wq[r0:r1].reshape(HPC, HD, D)[:, perm, :].reshape(CW, D)
        wk_c = wk[r0:r1].reshape(HPC, HD, D)[:, perm, :].reshape(CW, D)
        wv_c = wv[r0:r1]
        wo_c = wo[r0:r1]
        in_maps.append({
            "xT": xT,
            "wqT": np.ascontiguousarray(wq_c.T.astype(BF16)),
            "wkT": np.ascontiguousarray(wk_c.T.astype(BF16)),
            "wvT": np.ascontiguousarray(wv_c.T.astype(BF16)),
            "woT": np.ascontiguousarray(wo_c.T.astype(BF16)),
            "maskD": maskDb,
            "cq": cqh, "sq": sqh, "ck": ckh, "sk": skh,
        })
    return in_maps


def kernel(x, start_pos, freqs_cos, freqs_sin, mask, wq, wk, wv, wo,
           cache_k, cache_v, _trace=False):
    assert int(start_pos) == 0, "kernel specialized for start_pos=0"
    in_maps = _prep_inputs(x, freqs_cos, freqs_sin, mask, wq, wk, wv, wo)
    nc = _get_nc()
    res = run_bass_kernel_spmd(nc, in_maps, list(range(NCORES)), trace=_trace)
    kernel.last_results = res
    yfull = np.concatenate([res.results[c]["y"] for c in range(NCORES)],
                           axis=1)
    return yfull.reshape(B, S, D).astype(np.float32)


# revision 23
# speedup vs baseline: 1.0662x; 1.0662x over previous
"""Trainium2 Bass kernel for llama-style attention block (B=4, S=1024, D=4096, H=32).

Strategy: tensor-parallel over heads across 8 NeuronCores (4 heads/core).
 - Host marshals inputs: x transposed to [D, T] (contraction dim on
   partitions), per-core weight slices pre-transposed, q/k weight rows
   deinterleaved (even/odd RoPE pairs -> partition blocks [0:64]/[64:128]),
   everything matmul-facing cast to bf16.
 - Device per core: QKV projections (PE, fp32 accum) -> RoPE (DVE) ->
   attention in transposed layout S^T[k,q]: per k-tile the scores stream
   only the live causal q-range [128*kt, S); the additive mask is applied
   only on the 128-wide diagonal block; softmax denominators come from
   all-ones matmuls over the live ranges (partition reduction) and a
   fast-approx reciprocal; P@V accumulates live ranges into two PSUM
   banks. Slots (batch, head) are software-pipelined: scores(i) on the PE
   interleave with P@V/denominator(i-1), and wo token-pair GEMMs trail
   one batch behind their context AllGather, filling PE idle while ACT
   drains the softmax exps -> host concatenates per-core y columns.
 - Host concatenates the 8 per-core y column slices.

kernel(**inputs) takes the full unsharded inputs as in reference.setup_inputs()
and returns the full [4, 1024, 4096] float32 output.
"""

import math
import sys

import numpy as np
import ml_dtypes

sys.path.insert(0, "/opt/trn_rl_repo")

import concourse.bass as bass  # noqa: E402
import concourse.bass_isa as bass_isa  # noqa: E402
import concourse.mybir as mybir  # noqa: E402
import concourse.tile as tile  # noqa: E402
from concourse import bacc  # noqa: E402
from concourse.bass_utils import run_bass_kernel_spmd  # noqa: E402

P = 128
B, S, D, H = 4, 1024, 4096, 32
T = B * S
HD = 128
NCORES = 8
HPC = H // NCORES          # heads per core = 4
CW = HPC * HD              # per-core width = 512
NDK = D // P               # 32 contraction tiles
TCH = 256                  # token chunk in projection phase
NKT = S // P               # 8 k tiles per batch

MM = mybir.dt.bfloat16     # matmul operand dtype
F32 = mybir.dt.float32
BF16 = ml_dtypes.bfloat16

AG_GROUPS = [list(range(NCORES))]


def build_program():
    """Specialized to causal start_pos=0: per k-tile kt, queries in
    [128*kt, S) are live; the [128*kt, 128*(kt+1)) block needs the mask."""
    nc = bacc.Bacc("TRN2", target_bir_lowering=False, debug=False,
                   num_devices=NCORES)

    xT = nc.dram_tensor("xT", [D, T], MM, kind="ExternalInput").ap()
    wqT = nc.dram_tensor("wqT", [D, CW], MM, kind="ExternalInput").ap()
    wkT = nc.dram_tensor("wkT", [D, CW], MM, kind="ExternalInput").ap()
    wvT = nc.dram_tensor("wvT", [D, CW], MM, kind="ExternalInput").ap()
    woT = nc.dram_tensor("woT", [D, CW], MM, kind="ExternalInput").ap()
    maskD = nc.dram_tensor("maskD", [P, NKT * P], MM, kind="ExternalInput").ap()
    ident = nc.dram_tensor("ident", [P, P], MM, kind="ExternalInput").ap()
    cq = nc.dram_tensor("cq", [HD // 2, S], F32, kind="ExternalInput").ap()
    sq = nc.dram_tensor("sq", [HD // 2, S], F32, kind="ExternalInput").ap()
    ck = nc.dram_tensor("ck", [HD // 2, S], F32, kind="ExternalInput").ap()
    sk = nc.dram_tensor("sk", [HD // 2, S], F32, kind="ExternalInput").ap()
    y = nc.dram_tensor("y", [T, CW], F32, kind="ExternalOutput").ap()

    qT_d = nc.dram_tensor("qT_d", [CW, T], MM).ap()
    kT_d = nc.dram_tensor("kT_d", [CW, T], MM).ap()
    v_d = nc.dram_tensor("v_d", [T, CW], MM).ap()
    bounce = [nc.dram_tensor(f"bnc{i}", [CW, S], MM).ap()
              for i in range(B)]
    ctxT = [nc.dram_tensor(f"ctxT{i}", [D, S], MM,
                           addr_space="Shared").ap()
            for i in range(B)]

    sub = mybir.AluOpType.subtract
    add = mybir.AluOpType.add
    mult = mybir.AluOpType.mult
    Exp = mybir.ActivationFunctionType.Exp

    with tile.TileContext(nc) as tc:
        # long-lived pools: batch-0 attention prefetch + constants
        with tc.tile_pool(name="p0", bufs=1) as p0, \
             tc.tile_pool(name="mpool", bufs=1) as mpool:

            qb0 = p0.tile([P, HPC, S], MM)
            kb0 = p0.tile([P, HPC, S], MM)
            vb0 = p0.tile([P, NKT, CW], MM)
            mask_sb = mpool.tile([P, NKT, P], MM)
            ones_sb = mpool.tile([P, P], MM)
            id_sb = mpool.tile([P, P], MM)

            # ---------------- Phase A: projections + RoPE ----------------
            with tc.tile_pool(name="wpool", bufs=1) as wpool, \
                 tc.tile_pool(name="cspool", bufs=1) as cspool, \
                 tc.tile_pool(name="xpool", bufs=2) as xpool, \
                 tc.tile_pool(name="psa", bufs=4, space="PSUM") as psa, \
                 tc.tile_pool(name="stga", bufs=4) as stga, \
                 tc.tile_pool(name="tmpa", bufs=2) as tmpa:

                # first x chunk + head-0 q weights in interleaved pieces on
                # two DMA queues so the first accumulation group starts early
                x_first = xpool.tile([P, NDK, TCH], MM, tag="x")
                xTr = xT[:, 0:TCH].rearrange("(o p) t -> p o t", p=P)
                wq_sb = wpool.tile([P, NDK, CW], MM)
                wk_sb = wpool.tile([P, NDK, CW], MM)
                wv_sb = wpool.tile([P, NDK, CW], MM)
                wqr = wqT[:, 0:HD].rearrange("(o p) m -> p o m", p=P)
                wkr = wkT[:, 0:HD].rearrange("(o p) m -> p o m", p=P)
                for pc in range(8):
                    dsl = slice(pc * 4, (pc + 1) * 4)
                    nc.sync.dma_start(wq_sb[:, dsl, 0:HD], wqr[:, dsl, :])
                    nc.scalar.dma_start(x_first[:, dsl, :], xTr[:, dsl, :])
                nc.sync.dma_start(wk_sb[:, :, 0:HD], wkr)

                cq_sb = cspool.tile([HD // 2, S], F32)
                sq_sb = cspool.tile([HD // 2, S], F32)
                ck_sb = cspool.tile([HD // 2, S], F32)
                sk_sb = cspool.tile([HD // 2, S], F32)
                nc.sync.dma_start(cq_sb, cq)
                nc.sync.dma_start(sq_sb, sq)
                nc.sync.dma_start(ck_sb, ck)
                nc.sync.dma_start(sk_sb, sk)

                for h in range(1, HPC):
                    hs = slice(h * HD, (h + 1) * HD)
                    nc.sync.dma_start(
                        wq_sb[:, :, hs],
                        wqT[:, hs].rearrange("(o p) m -> p o m", p=P))
                    nc.sync.dma_start(
                        wk_sb[:, :, hs],
                        wkT[:, hs].rearrange("(o p) m -> p o m", p=P))
                nc.sync.dma_start(wv_sb, wvT.rearrange("(o p) m -> p o m", p=P))

                for tch in range(T // TCH):
                    t0 = tch * TCH
                    s0 = t0 % S
                    if tch == 0:
                        x_sb = x_first
                    else:
                        # x on the scalar DMA queue: keeps the token stream
                        # off the weight-load queue so chunks never stall
                        x_sb = xpool.tile([P, NDK, TCH], MM, tag="x")
                        nc.scalar.dma_start(
                            x_sb,
                            xT[:, t0:t0 + TCH].rearrange("(o p) t -> p o t",
                                                         p=P))

                    # q/k for the 4 local heads; RoPE on psum eviction
                    for h in range(HPC):
                        for wsb, c_sb, s_sb, dst in (
                                (wq_sb, cq_sb, sq_sb, qT_d),
                                (wk_sb, ck_sb, sk_sb, kT_d)):
                            ps = psa.tile([P, TCH], F32, tag="qk")
                            for dk in range(NDK):
                                nc.tensor.matmul(
                                    ps, lhsT=wsb[:, dk, h * HD:(h + 1) * HD],
                                    rhs=x_sb[:, dk, :],
                                    start=(dk == 0), stop=(dk == NDK - 1))
                            a = ps[0:HD // 2]
                            bb = ps[HD // 2:P]
                            cc = c_sb[:, s0:s0 + TCH]
                            ss = s_sb[:, s0:s0 + TCH]
                            t1 = tmpa.tile([HD // 2, TCH], F32, tag="t1")
                            t2 = tmpa.tile([HD // 2, TCH], F32, tag="t2")
                            t3 = tmpa.tile([HD // 2, TCH], F32, tag="t3")
                            t4 = tmpa.tile([HD // 2, TCH], F32, tag="t4")
                            out = stga.tile([P, TCH], MM, tag="qkstage")
                            nc.vector.tensor_tensor(t1, a, cc, mult)
                            nc.vector.tensor_tensor(t2, bb, ss, mult)
                            nc.vector.tensor_tensor(out[0:HD // 2], t1, t2, sub)
                            nc.vector.tensor_tensor(t3, a, ss, mult)
                            nc.vector.tensor_tensor(t4, bb, cc, mult)
                            nc.vector.tensor_tensor(out[HD // 2:P], t3, t4, add)
                            nc.sync.dma_start(
                                dst[h * HD:(h + 1) * HD, t0:t0 + TCH], out)

                    # v for the 4 local heads (natural [t, hd] layout);
                    # evict on the otherwise-idle scalar engine
                    for tt in range(TCH // P):
                        ps = psa.tile([P, CW], F32, tag="v")
                        for dk in range(NDK):
                            nc.tensor.matmul(
                                ps, lhsT=x_sb[:, dk, tt * P:(tt + 1) * P],
                                rhs=wv_sb[:, dk, :],
                                start=(dk == 0), stop=(dk == NDK - 1))
                        vo = stga.tile([P, CW], MM, tag="vstage")
                        nc.scalar.copy(vo, ps)
                        nc.sync.dma_start(
                            v_d[t0 + tt * P:t0 + (tt + 1) * P, :], vo)

                    # prefetch batch-0 attention tiles once its q/k/v stored
                    if tch == S // TCH - 1:
                        nc.sync.dma_start(
                            qb0, qT_d[:, 0:S]
                            .rearrange("(h p) t -> p h t", p=P))
                        nc.sync.dma_start(
                            kb0, kT_d[:, 0:S]
                            .rearrange("(h p) t -> p h t", p=P))
                        nc.sync.dma_start(
                            vb0, v_d[0:S, :]
                            .rearrange("(kt p) w -> p kt w", p=P))
                        nc.sync.dma_start(
                            mask_sb,
                            maskD.rearrange("p (kt q) -> p kt q", q=P))
                        nc.sync.dma_start(id_sb, ident)
                        nc.any.memset(ones_sb, 1.0)

            # ------------- Phase B/C: attention + AllGather + wo ---------
            with tc.tile_pool(name="qkvp", bufs=2) as qkvp, \
                 tc.tile_pool(name="esp", bufs=2) as esp, \
                 tc.tile_pool(name="psb", bufs=1, space="PSUM") as psb, \
                 tc.tile_pool(name="tmpb", bufs=4) as tmpb, \
                 tc.tile_pool(name="stgb", bufs=4) as stgb, \
                 tc.tile_pool(name="wop", bufs=1) as wop, \
                 tc.tile_pool(name="cxp", bufs=2) as cxp:

                wo_sb = wop.tile([P, NDK, CW], MM)
                nc.sync.dma_start(wo_sb,
                                  woT.rearrange("(o p) m -> p o m", p=P))

                def load_batch(b):
                    qb = qkvp.tile([P, HPC, S], MM, tag="qb")
                    kb = qkvp.tile([P, HPC, S], MM, tag="kb")
                    vb = qkvp.tile([P, NKT, CW], MM, tag="vb")
                    nc.sync.dma_start(
                        qb, qT_d[:, b * S:(b + 1) * S]
                        .rearrange("(h p) t -> p h t", p=P))
                    nc.sync.dma_start(
                        kb, kT_d[:, b * S:(b + 1) * S]
                        .rearrange("(h p) t -> p h t", p=P))
                    nc.sync.dma_start(
                        vb, v_d[b * S:(b + 1) * S, :]
                        .rearrange("(kt p) w -> p kt w", p=P))
                    return qb, kb, vb

                def pv_mm(st, kt):
                    # P@V + denominator matmuls for the previous slot,
                    # interleaved between the current slot's score MMs.
                    # Denominator MMs (all-ones stationary) are compressed
                    # into kt 0..3 so the reciprocal can run mid-slot.
                    b, h, qb, kb, vb, es, ps_o0, ps_o1, ps_d0, ps_d1 = st
                    hs = slice(h * HD, (h + 1) * HD)
                    if kt < 4:
                        lo = P * kt
                        nc.tensor.matmul(
                            ps_o0[:, lo:512], lhsT=vb[:, kt, hs],
                            rhs=es[:, kt, lo:512],
                            start=(kt == 0), stop=(kt == 3),
                            skip_group_check=True)
                        for kd in (2 * kt, 2 * kt + 1):
                            lo_d = P * kd
                            if kd < 4:
                                nc.tensor.matmul(
                                    ps_d0[:, lo_d:512], lhsT=ones_sb,
                                    rhs=es[:, kd, lo_d:512],
                                    start=(kd == 0), stop=(kd == 3),
                                    skip_group_check=True)
                            lo2_d = max(512, lo_d)
                            nc.tensor.matmul(
                                ps_d1[:, lo2_d - 512:512], lhsT=ones_sb,
                                rhs=es[:, kd, lo2_d:S],
                                start=(kd == 0), stop=(kd == NKT - 1),
                                skip_group_check=True)
                    lo2 = max(512, P * kt)
                    nc.tensor.matmul(
                        ps_o1[:, lo2 - 512:512], lhsT=vb[:, kt, hs],
                        rhs=es[:, kt, lo2:S],
                        start=(kt == 0), stop=(kt == NKT - 1),
                        skip_group_check=True)

                def rcp_prev(st):
                    b, h = st[0], st[1]
                    ps_d0, ps_d1 = st[8], st[9]
                    rec = tmpb.tile([P, S], F32, tag="rec", bufs=2)
                    nc.vector.reciprocal_approx_fast(rec[:, 0:512], ps_d0)
                    nc.vector.reciprocal_approx_fast(rec[:, 512:S], ps_d1)
                    return rec

                def finish_prev(st, rec):
                    b, h = st[0], st[1]
                    ps_o0, ps_o1 = st[6], st[7]
                    for q2, ps_o in ((0, ps_o0), (1, ps_o1)):
                        ob = stgb.tile([P, 512], MM, tag="ob", bufs=3)
                        nc.vector.tensor_tensor(
                            ob, ps_o, rec[:, q2 * 512:(q2 + 1) * 512], mult)
                        nc.sync.dma_start(
                            bounce[b][h * HD:(h + 1) * HD,
                                      q2 * 512:(q2 + 1) * 512], ob)

                def allgather(i):
                    nc.gpsimd.collective_compute(
                        "AllGather", mybir.AluOpType.bypass,
                        replica_groups=AG_GROUPS,
                        ins=[bounce[i]], outs=[ctxT[i]])

                def wo_pair(b, pp):
                    # one token-pair (256 tokens) of the wo projection,
                    # interleaved between attention slots so the PE-bound
                    # wo GEMM overlaps the ACT-bound softmax
                    c0 = 2 * pp * P
                    cx = cxp.tile([P, NDK, 2 * P], MM, tag="cx")
                    nc.scalar.dma_start(
                        cx, ctxT[b][:, c0:c0 + 2 * P]
                        .rearrange("(o p) t -> p o t", p=P))
                    ps_y0 = psb.tile([P, CW], F32, tag="y", bufs=2)
                    ps_y1 = psb.tile([P, CW], F32, tag="y", bufs=2)
                    for dk in range(NDK):
                        nc.tensor.matmul(
                            ps_y0, lhsT=cx[:, dk, 0:P],
                            rhs=wo_sb[:, dk, :],
                            start=(dk == 0), stop=(dk == NDK - 1))
                        nc.tensor.matmul(
                            ps_y1, lhsT=cx[:, dk, P:2 * P],
                            rhs=wo_sb[:, dk, :],
                            start=(dk == 0), stop=(dk == NDK - 1))
                    for j, ps_y in enumerate((ps_y0, ps_y1)):
                        yo = stgb.tile([P, CW], F32, tag="yo", bufs=2)
                        nc.scalar.copy(yo, ps_y)
                        nc.sync.dma_start(
                            y[b * S + (2 * pp + j) * P:
                              b * S + (2 * pp + j + 1) * P, :], yo)

                def emit_slot(cur, prev):
                    # score MMs of `cur` with PV/den of `prev` interleaved;
                    # the causal mask on the diagonal block is folded into
                    # the PE as an identity-rhs accumulate matmul
                    b, h, qb, kb, vb, es = cur[:6]
                    rec = None
                    for kt in range(NKT):
                        lo = P * kt
                        chunks = [(lo, min(lo + 512, S))]
                        if S - lo > 512:
                            chunks.append((lo + 512, S))
                        for (c0, c1) in chunks:
                            n = c1 - c0
                            diag = c0 == lo
                            ps_s = psb.tile([P, 512], F32, tag="sc", bufs=2)
                            nc.tensor.matmul(
                                ps_s[:, 0:n], lhsT=kb[:, h, lo:lo + P],
                                rhs=qb[:, h, c0:c1],
                                start=True, stop=not diag,
                                skip_group_check=True)
                            if diag:
                                nc.tensor.matmul(
                                    ps_s[:, 0:P], lhsT=id_sb,
                                    rhs=mask_sb[:, kt, :],
                                    start=False, stop=True,
                                    skip_group_check=True)
                            nc.scalar.activation(
                                es[:, kt, c0:c1], ps_s[:, 0:n], Exp)
                        if prev is not None:
                            pv_mm(prev, kt)
                            if kt == 4:
                                rec = rcp_prev(prev)
                    if prev is not None:
                        if rec is None:
                            rec = rcp_prev(prev)
                        finish_prev(prev, rec)
                        if prev[1] == 3:
                            allgather(prev[0])

                # flat head pipeline over (batch, head) slots; wo batches
                # trail their AllGather by one batch and interleave between
                # slots to keep the PE dense while ACT drains softmax
                prev = None
                batch_tiles = {0: (qb0, kb0, vb0)}
                for i in range(B * HPC):
                    b, h = divmod(i, HPC)
                    if h == 3 and b + 1 < B:
                        batch_tiles[b + 1] = load_batch(b + 1)
                    qb, kb, vb = batch_tiles[b]
                    es = esp.tile([P, NKT, S], MM, tag="es")
                    cur = (b, h, qb, kb, vb, es, None, None, None, None)
                    if prev is not None:
                        ps_o0 = psb.tile([P, 512], F32, tag="o0", bufs=1)
                        ps_o1 = psb.tile([P, 512], F32, tag="o1", bufs=1)
                        ps_d0 = psb.tile([P, 512], F32, tag="d0", bufs=1)
                        ps_d1 = psb.tile([P, 512], F32, tag="d1", bufs=1)
                        prev = prev[:6] + (ps_o0, ps_o1, ps_d0, ps_d1)
                    emit_slot(cur, prev)
                    # wo trails its AllGather by two batches (the collective
                    # has ~35us of startup latency on the CC engine)
                    if b >= 2 and h >= 1:
                        wo_pair(b - 2, h - 1)
                    if b >= 2 and h == 3:
                        wo_pair(b - 2, 3)
                    prev = cur
                # drain the last slot, then the final wo batch
                ps_o0 = psb.tile([P, 512], F32, tag="o0", bufs=1)
                ps_o1 = psb.tile([P, 512], F32, tag="o1", bufs=1)
                ps_d0 = psb.tile([P, 512], F32, tag="d0", bufs=1)
                ps_d1 = psb.tile([P, 512], F32, tag="d1", bufs=1)
                prev = prev[:6] + (ps_o0, ps_o1, ps_d0, ps_d1)
                for kt in range(NKT):
                    pv_mm(prev, kt)
                rec = rcp_prev(prev)
                finish_prev(prev, rec)
                allgather(prev[0])
                for pp in range(4):
                    wo_pair(B - 2, pp)
                for pp in range(4):
                    wo_pair(B - 1, pp)

    nc.compile()
    return nc


_NC_CACHE = {}


def _get_nc():
    if "nc" not in _NC_CACHE:
        _NC_CACHE["nc"] = build_program()
    return _NC_CACHE["nc"]


def _check_causal(maskT_f32):
    """Verify the mask is standard causal (start_pos=0): per k-tile kt,
    queries < 128*kt are fully masked, the diagonal block is the only
    partially-masked one, and everything beyond is unmasked."""
    for kt in range(NKT):
        lo = P * kt
        t = maskT_f32[lo:lo + P, :]
        if lo > 0 and not np.all(t[:, :lo] <= -1e30):
            return False
        if not np.all(t[:, lo + P:] == 0.0):
            return False
    return True


def _prep_inputs(x, freqs_cos, freqs_sin, mask, wq, wk, wv, wo):
    """Host-side sharding/layout marshaling. Returns per-core input maps."""
    x = np.asarray(x, np.float32).reshape(T, D)
    xT = np.ascontiguousarray(x.T.astype(BF16))

    cos = np.asarray(freqs_cos, np.float32)
    sin = np.asarray(freqs_sin, np.float32)
    qscale = 1.0 / math.sqrt(HD)
    cqh = np.ascontiguousarray(cos.T * qscale).astype(np.float32)
    sqh = np.ascontiguousarray(sin.T * qscale).astype(np.float32)
    ckh = np.ascontiguousarray(cos.T).astype(np.float32)
    skh = np.ascontiguousarray(sin.T).astype(np.float32)

    m = np.asarray(mask, np.float32).reshape(S, S)
    mT = np.ascontiguousarray(m.T)
    assert _check_causal(mT), "kernel specialized for causal start_pos=0"
    # diagonal blocks of mask^T, clamped for bf16
    maskDh = np.empty((P, NKT * P), np.float32)
    for kt in range(NKT):
        lo = P * kt
        maskDh[:, lo:lo + P] = mT[lo:lo + P, lo:lo + P]
    maskDb = np.ascontiguousarray(np.maximum(maskDh, -60000.0).astype(BF16))
    identb = np.ascontiguousarray(np.eye(P, dtype=np.float32).astype(BF16))

    # deinterleave RoPE pairs within each head's weight rows: row order
    # [0,2,...,126,1,3,...,127] so pairs land in partition blocks.
    perm = np.concatenate([np.arange(0, HD, 2), np.arange(1, HD, 2)])

    wq = np.asarray(wq, np.float32)
    wk = np.asarray(wk, np.float32)
    wv = np.asarray(wv, np.float32)
    wo = np.asarray(wo, np.float32)

    in_maps = []
    for c in range(NCORES):
        r0, r1 = c * CW, (c + 1) * CW
        wq_c = wq[r0:r1].reshape(HPC, HD, D)[:, perm, :].reshape(CW, D)
        wk_c = wk[r0:r1].reshape(HPC, HD, D)[:, perm, :].reshape(CW, D)
        wv_c = wv[r0:r1]
        wo_c = wo[r0:r1]
        in_maps.append({
            "xT": xT,
            "wqT": np.ascontiguousarray(wq_c.T.astype(BF16)),
            "wkT": np.ascontiguousarray(wk_c.T.astype(BF16)),
            "wvT": np.ascontiguousarray(wv_c.T.astype(BF16)),
            "woT": np.ascontiguousarray(wo_c.T.astype(BF16)),
            "maskD": maskDb, "ident": identb,
            "cq": cqh, "sq": sqh, "ck": ckh, "sk": skh,
        })
    return in_maps


def kernel(x, start_pos, freqs_cos, freqs_sin, mask, wq, wk, wv, wo,
           cache_k, cache_v, _trace=False):
    assert int(start_pos) == 0, "kernel specialized for start_pos=0"
    in_maps = _prep_inputs(x, freqs_cos, freqs_sin, mask, wq, wk, wv, wo)
    nc = _get_nc()
    res = run_bass_kernel_spmd(nc, in_maps, list(range(NCORES)), trace=_trace)
    kernel.last_results = res
    yfull = np.concatenate([res.results[c]["y"] for c in range(NCORES)],
                           axis=1)
    return yfull.reshape(B, S, D).astype(np.float32)
